# revision 61
# baseline (speedup 1.0000x reference)
"""Trainium2 Bass kernel: LeViT-style attention block (B=256, C=384, 14x14, 8 heads).

Data-parallel over batch: 32 images per NeuronCore, 8 cores.
Self-contained: takes full inputs, shards, runs SPMD, gathers full output.

v2: softmax ab-multiply on GpSimd, rowsum via DVE reduce, softmax
normalization fused into the PE transpose via diag(1/rowsum) as the
transpose multiplier, per-head merged PT/relu tiles.
"""
import os
import sys
import types

import numpy as np
import ml_dtypes

import concourse.bacc as bacc
import concourse.tile as tile
from concourse import mybir
from concourse.bass_utils import run_bass_kernel_spmd
from concourse.masks import make_identity

BF16 = ml_dtypes.bfloat16
EPS = 1e-5
NCORES = 8
B = 256
BPC = B // NCORES          # 32 images per core
PAIRS = BPC // 2
DIM, KEY_DIM, HEADS, RES = 384, 32, 8, 14
N = RES * RES              # 196
NH_KD, D, DH, H_QKV = 256, 128, 1024, 1536
SCALE = KEY_DIM ** -0.5
DT = mybir.dt
AF = mybir.ActivationFunctionType
OP = mybir.AluOpType

LAST_RESULT = None
_NC_CACHE = {}


def _install_ntff_hook():
    # The image's antenv lacks axon_hooks; synthesize it so trace=True (or
    # BASS_TRACE=1) yields exec_time_ns via the ctypes NTFF hook.
    try:
        import antenv
        from trn_agent_boot.trn_boot import _ntff_profile_via_ctypes

        if "antenv.axon_hooks" in sys.modules:
            return
        mod = types.ModuleType("antenv.axon_hooks")
        mod._hook = _ntff_profile_via_ctypes("/opt/axon/libaxon_pjrt.so")
        mod.set_axon_ntff_profile_hook = lambda h: setattr(mod, "_hook", h)
        mod.get_axon_ntff_profile_hook = lambda: mod._hook
        sys.modules["antenv.axon_hooks"] = mod
        antenv.axon_hooks = mod
    except Exception:
        pass


_install_ntff_hook()


def _build_nc():
    nc = bacc.Bacc("TRN2", target_bir_lowering=False, debug=False)
    x_d = nc.declare_dram_parameter("x", [BPC, DIM, N], DT.bfloat16, isOutput=False)
    qkvw_d = nc.declare_dram_parameter("qkv_wT", [DIM, H_QKV], DT.bfloat16, isOutput=False)
    dww_d = nc.declare_dram_parameter("dw_w", [2, 9, 128], DT.bfloat16, isOutput=False)
    projw_d = nc.declare_dram_parameter("proj_wT", [DH, DIM], DT.bfloat16, isOutput=False)
    qkvb_d = nc.declare_dram_parameter("qkv_bias", [12, 128], DT.float32, isOutput=False)
    dwb_d = nc.declare_dram_parameter("dw_bias", [2, 128], DT.float32, isOutput=False)
    projb_d = nc.declare_dram_parameter("proj_bias", [3, 128], DT.float32, isOutput=False)
    ab_d = nc.declare_dram_parameter("ab", [HEADS, N, N], DT.bfloat16, isOutput=False)
    out_d = nc.declare_dram_parameter("out", [BPC, DIM, N], DT.float32, isOutput=True)

    from contextlib import ExitStack

    with tile.TileContext(nc) as tc, ExitStack() as es:
        const = es.enter_context(tc.tile_pool(name="const", bufs=1))
        xin = es.enter_context(tc.tile_pool(name="xin", bufs=4))
        stage = es.enter_context(tc.tile_pool(name="stage", bufs=4))
        vtp = es.enter_context(tc.tile_pool(name="vtp", bufs=6))
        sm = es.enter_context(tc.tile_pool(name="sm", bufs=8))
        att = es.enter_context(tc.tile_pool(name="att", bufs=12))
        outp = es.enter_context(tc.tile_pool(name="outp", bufs=4))
        psum = es.enter_context(tc.tile_pool(name="psum", bufs=2, space="PSUM"))

        # ---- constants (ordered by first use so the pipeline ramps fast) ----
        qkvw_sb = const.tile([128, 3, H_QKV], DT.bfloat16)
        qkvb_sb = const.tile([128, 12], DT.float32)
        nc.sync.dma_start(qkvb_sb[:], qkvb_d.ap().rearrange("m q -> q m"))
        # q,k weight columns feed the very first matmuls; v columns can land later.
        # Chunked across DMA rings (~22GB/s each) to shorten the ramp.
        for c0 in range(0, 512, 128):
            nc.sync.dma_start(
                qkvw_sb[:, :, c0:c0 + 128],
                qkvw_d.ap()[:, c0:c0 + 128].rearrange("(k q) m -> q k m", q=128))
        dwb_sb = const.tile([128, 2], DT.float32)
        dww_sb = const.tile([128, 2, 9, 1], DT.bfloat16)
        dwdiag_sb = const.tile([128, 2, 9, 128], DT.bfloat16)
        ident = const.tile([128, 1, 128], DT.bfloat16)
        make_identity(nc, ident[:, 0])

        def mid_consts():
            # emitted after pair 0's x loads: needed only a few microseconds in
            for c0 in range(512, H_QKV, 256):
                nc.sync.dma_start(
                    qkvw_sb[:, :, c0:c0 + 256],
                    qkvw_d.ap()[:, c0:c0 + 256].rearrange("(k q) m -> q k m", q=128))
            nc.sync.dma_start(dwb_sb[:], dwb_d.ap().rearrange("m q -> q m"))
            # dw diag matrices built on-chip from the small [2,9,128] weight table
            nc.sync.dma_start(dww_sb[:, :, :, 0], dww_d.ap().rearrange("c t q -> q c t"))
            for c2 in range(2):
                nc.gpsimd.affine_select(
                    dwdiag_sb[:, c2], dww_sb[:, c2].broadcast_to([128, 9, 128]),
                    pattern=[[0, 9], [-1, 128]], compare_op=OP.is_equal,
                    fill=0.0, base=0, channel_multiplier=1)

        def late_consts():
            # stage_B/C constants: landed while pair 0's stage_A runs
            nc.sync.dma_start(ab_sb[:, 0], ab_d.ap()[:, 0:128].rearrange("h q m -> q h m"))
            nc.sync.dma_start(ab_sb[0:68, 1], ab_d.ap()[:, 128:196].rearrange("h q m -> q h m"))
            nc.sync.dma_start(projw_sb[:], projw_d.ap().rearrange("(k q) m -> q k m", q=128))
            nc.sync.dma_start(projb_sb[:], projb_d.ap().rearrange("m q -> q m"))

        # combined exp(ab) table: [:, 0, h, :] = rows 0..127, [:68, 1, h, :] rows 128..195
        ab_sb = const.tile([128, 2, HEADS, N], DT.bfloat16)
        nc.gpsimd.memset(ab_sb[:, 1].rearrange("q h m -> q (h m)"), 0.0)
        projw_sb = const.tile([128, 8, DIM], DT.bfloat16)
        projb_sb = const.tile([128, 3], DT.float32)

        def stage_A(p, pipe):
            """Load x pair; qkv (q,k) + v^T-direct + depthwise conv."""
            i0 = 2 * p
            xt = xin.tile([128, 3, 2, N], DT.bfloat16)
            if p == 0:
                # 3 triggers at startup: spread across rings for low latency
                for k in range(3):
                    nc.sync.dma_start(
                        xt[:, k],
                        x_d.ap()[i0:i0 + 2, 128 * k:128 * (k + 1)].rearrange("i q n -> q i n"),
                    )
            else:
                # steady state: two triggers save Sync-sequencer issue time
                for i in range(2):
                    nc.sync.dma_start(
                        xt[:, :, i],
                        x_d.ap()[i0 + i].rearrange("(k q) n -> q k n", q=128),
                    )
            qp = stage.tile([128, 2, 2, 16, 16], DT.bfloat16, tag="qp")
            nc.gpsimd.memset(qp[:], 0.0)
            k_sb = stage.tile([128, 2, 2, N], DT.bfloat16, tag="k_sb")
            pipe["xt"], pipe["qp"], pipe["k_sb"] = xt, qp, k_sb
            yield
            for m in range(4):
                ps = psum.tile([128, 2, N], DT.float32, tag="mm")
                for k in range(3):
                    nc.tensor.matmul(
                        ps[:], lhsT=qkvw_sb[:, k, 128 * m:128 * (m + 1)],
                        rhs=xt[:, k], start=(k == 0), stop=(k == 2))
                bias = qkvb_sb[:, m:m + 1]
                if m < 2:
                    nc.scalar.activation(
                        qp[:, m, :, 1:15, 1:15],
                        ps[:].rearrange("q i (y x) -> q i y x", y=RES),
                        AF.Identity, bias=bias)
                else:
                    nc.scalar.activation(k_sb[:, m - 2], ps[:], AF.Identity, bias=bias)
                yield
            # v^T direct: vT[m, dv] = x^T @ Wv^T (v bias folded into O eviction)
            vTs = []
            for img in range(2):
                vT_sb = vtp.tile([128, 2, DH], DT.bfloat16, tag="vT_sb")
                for mc in range(2):
                    m_lo, m_sz = (0, 128) if mc == 0 else (128, 68)
                    for half in range(2):
                        pv = psum.tile([128, 512], DT.float32, tag="mm")
                        for k in range(3):
                            nc.tensor.matmul(
                                pv[0:m_sz],
                                lhsT=xt[:, k, img, m_lo:m_lo + m_sz],
                                rhs=qkvw_sb[:, k, 512 + 512 * half:512 + 512 * (half + 1)],
                                start=(k == 0), stop=(k == 2))
                        if half == 0:
                            nc.scalar.activation(
                                vT_sb[0:m_sz, mc, 0:512], pv[0:m_sz], AF.Copy)
                        else:
                            nc.vector.tensor_copy(
                                vT_sb[0:m_sz, mc, 512:1024], pv[0:m_sz])
                        yield
                vTs.append(vT_sb)
            pipe["vTs"] = vTs
            # depthwise 3x3 conv as 9 diagonal matmuls over padded 16x16
            qdw_sb = stage.tile([128, 2, 2, N], DT.bfloat16, tag="qdw_sb")
            pipe["qdw"] = qdw_sb
            for c2 in range(2):
                pd = psum.tile([128, 2, RES, RES], DT.float32, tag="mm")
                for tap in range(9):
                    dy, dx = divmod(tap, 3)
                    nc.tensor.matmul(
                        pd[:], lhsT=dwdiag_sb[:, c2, tap],
                        rhs=qp[:, c2, :, dy:dy + 14, dx:dx + 14],
                        start=(tap == 0), stop=(tap == 8))
                    if tap % 3 == 2:
                        yield
                nc.vector.tensor_scalar_add(
                    qdw_sb[:, c2].rearrange("q i (y x) -> q i y x", y=RES),
                    pd[:], dwb_sb[:, c2:c2 + 1])
                yield

        def stage_B(p, pipe):
            """Attention: S = q^T k; P = exp(S)*exp_ab (GpSimd mult, DVE rowsum);
            P^T via PE transpose against diag(1/rowsum); O = v @ P^T; relu."""
            k_sb, qdw_sb, vTs = pipe["k_sb"], pipe["qdw"], pipe["vTs"]
            relu_sb = stage.tile([128, 8, 2, N], DT.bfloat16, tag="relu_sb")
            pipe["relu"] = relu_sb

            def s_matmul(img, h, Ss):
                ch, sub = divmod(h, 4)
                r0 = sub * 32
                q_ap = qdw_sb[r0:r0 + 32, ch, img]
                k_ap = k_sb[r0:r0 + 32, ch, img]
                S = psum.tile([128, 2, N], DT.float32, tag="S", bufs=2)
                nc.tensor.matmul(S[:, 0], lhsT=q_ap[:, 0:128], rhs=k_ap,
                                 start=True, stop=True, tile_position=(r0, 0))
                nc.tensor.matmul(S[:68, 1], lhsT=q_ap[:, 128:196], rhs=k_ap,
                                 start=True, stop=True, tile_position=(r0, 0))
                Ss[(img, h)] = S

            def softmax1(img, h, S, ssums, Ps):
                E = sm.tile([128, 2, N], DT.bfloat16, tag="E", bufs=4)
                nc.scalar.activation(E[:], S[:], AF.Exp)
                P = att.tile([128, 2, N], DT.bfloat16, tag="P", bufs=12)
                nc.vector.scalar_tensor_tensor(
                    P[:, 0], E[:, 0], 0.0, ab_sb[:, 0, h, :],
                    op0=OP.add, op1=OP.mult, accum_out=ssums[img][:, h, 0:1])
                nc.vector.scalar_tensor_tensor(
                    P[:68, 1], E[:68, 1], 0.0, ab_sb[:68, 1, h, :],
                    op0=OP.add, op1=OP.mult, accum_out=ssums[img][:68, h, 1:2])
                Ps[(img, h)] = P

            def make_diags(img, hg, rinv, diags):
                # diag(rinv) for 4 heads at once; bf16 for the PE transpose.
                # GpSimd is idle and the skew-4 pipeline hides its latency.
                d128 = att.tile([128, 4, 128], DT.bfloat16, tag="d128", bufs=4)
                nc.vector.tensor_tensor(
                    d128[:], ident[:, 0:1, :].broadcast_to([128, 4, 128]),
                    rinv[:, 4 * hg:4 * hg + 4, 0:1].broadcast_to([128, 4, 128]),
                    op=OP.mult)
                d68 = att.tile([68, 4, 68], DT.bfloat16, tag="d68", bufs=4)
                nc.vector.tensor_tensor(
                    d68[:], ident[0:68, 0:1, 0:68].broadcast_to([68, 4, 68]),
                    rinv[0:68, 4 * hg:4 * hg + 4, 1:2].broadcast_to([68, 4, 68]),
                    op=OP.mult)
                diags[(img, hg)] = (d128, d68)

            def phase2(h, diags, Ps):
                hg, hs = divmod(h, 4)
                PT = att.tile([128, 2, 2, N], DT.bfloat16, tag="PT_sb", bufs=3)
                for img in range(2):
                    P = Ps[(img, h)]
                    d128, d68 = diags[(img, hg)]
                    # normalized transpose: PTp = P^T @ diag(rinv) (real matmul;
                    # is_transpose mode ignores the multiplier values)
                    PTp = psum.tile([128, 2, N], DT.float32, tag="PT", bufs=2)
                    nc.tensor.matmul(PTp[:, 0, 0:128], lhsT=P[:, 0, 0:128],
                                     rhs=d128[:, hs], start=True, stop=True)
                    nc.tensor.matmul(PTp[:68, 1, 0:128], lhsT=P[:, 0, 128:196],
                                     rhs=d128[:, hs], start=True, stop=True)
                    nc.tensor.matmul(PTp[:, 0, 128:196], lhsT=P[:68, 1, 0:128],
                                     rhs=d68[:, hs], start=True, stop=True)
                    nc.tensor.matmul(PTp[:68, 1, 128:196], lhsT=P[:68, 1, 128:196],
                                     rhs=d68[:, hs], start=True, stop=True)
                    if img == 0:
                        nc.scalar.activation(PT[:, :, img, :], PTp[:], AF.Copy)
                    else:
                        nc.vector.tensor_copy(PT[:, :, img, :], PTp[:])
                O = psum.tile([128, 2, N], DT.float32, tag="O", bufs=1)
                for img in range(2):
                    nc.tensor.matmul(O[:, img],
                                     lhsT=vTs[img][:, 0, 128 * h:128 * (h + 1)],
                                     rhs=PT[:, 0, img], start=True, stop=False)
                    nc.tensor.matmul(O[:, img],
                                     lhsT=vTs[img][:68, 1, 128 * h:128 * (h + 1)],
                                     rhs=PT[:68, 1, img], start=False, stop=True)
                nc.scalar.activation(relu_sb[:, h], O[:], AF.Relu,
                                     bias=qkvb_sb[:, 4 + h:5 + h])

            ssum0 = sm.tile([128, 8, 2], DT.float32, tag="ssum0")
            ssum1 = sm.tile([128, 8, 2], DT.float32, tag="ssum1")
            rinv0 = sm.tile([128, 8, 2], DT.float32, tag="rinv0")
            rinv1 = sm.tile([128, 8, 2], DT.float32, tag="rinv1")
            ssums = [ssum0, ssum1]
            rinvs = [rinv0, rinv1]
            Ps = {}
            Ss = {}
            diags = {}
            for h in range(HEADS):
                for img in range(2):
                    s_matmul(img, h, Ss)
                yield
                if h >= 4:
                    phase2(h - 4, diags, Ps)
                for img in range(2):
                    softmax1(img, h, Ss[(img, h)], ssums, Ps)
                if h % 4 == 3:
                    hg = h // 4
                    for img in range(2):
                        nc.vector.reciprocal(
                            rinvs[img][:, 4 * hg:4 * hg + 4].rearrange("q a b -> q (a b)"),
                            ssums[img][:, 4 * hg:4 * hg + 4].rearrange("q a b -> q (a b)"))
                        make_diags(img, hg, rinvs[img], diags)
                yield
            for h in range(HEADS - 4, HEADS):
                phase2(h, diags, Ps)
                yield

        def stage_C(p, pipe):
            """proj 1x1 conv (+BN fold) and output DMA."""
            i0 = 2 * p
            relu_sb = pipe["relu"]
            for m3 in range(3):
                pp = psum.tile([128, 2, N], DT.float32, tag="mmC", bufs=1)
                for k8 in range(8):
                    nc.tensor.matmul(
                        pp[:], lhsT=projw_sb[:, k8, 128 * m3:128 * (m3 + 1)],
                        rhs=relu_sb[:, k8], start=(k8 == 0), stop=(k8 == 7))
                ob = outp.tile([128, 2, N], DT.float32)
                nc.vector.tensor_scalar_add(ob[:], pp[:], projb_sb[:, m3:m3 + 1])
                nc.sync.dma_start(
                    out_d.ap()[i0:i0 + 2, 128 * m3:128 * (m3 + 1)].rearrange("i q n -> q i n"),
                    ob[:])
                yield

        # ---- 3-deep software pipeline: A(p) || B(p-1) || C(p-2) ----
        pipes = {}

        def drain(gens, weights=None):
            pairs = [(g, (weights or {}).get(i, 1)) for i, g in enumerate(gens)
                     if g is not None]
            while pairs:
                for entry in list(pairs):
                    g, w = entry
                    for _ in range(w):
                        try:
                            next(g)
                        except StopIteration:
                            pairs.remove(entry)
                            break

        for p in range(PAIRS):
            if p == 1:
                late_consts()
            pipes[p] = {}
            gA = stage_A(p, pipes[p])
            if p == 0:
                next(gA)      # emit pair-0 x loads before remaining consts
                mid_consts()
            gB = stage_B(p - 1, pipes[p - 1]) if p >= 1 else None
            gC = stage_C(p - 2, pipes[p - 2]) if p >= 2 else None
            drain([gA, gC, gB])
        drain([stage_B(PAIRS - 1, pipes[PAIRS - 1]),
               stage_C(PAIRS - 2, pipes[PAIRS - 2])])
        drain([stage_C(PAIRS - 1, pipes[PAIRS - 1])])

    nc.finalize()
    return nc


def _get_nc():
    if "nc" not in _NC_CACHE:
        _NC_CACHE["nc"] = _build_nc()
    return _NC_CACHE["nc"]


def _prep_host(x, qkv_w, qkv_g, qkv_b, qkv_m, qkv_v,
               dw_w, dw_g, dw_b, dw_m, dw_v,
               proj_w, proj_g, proj_b, proj_m, proj_v,
               attention_biases, bias_idxs):
    f = np.float32
    x = np.asarray(x, f)
    s = np.asarray(qkv_g, f) / np.sqrt(np.asarray(qkv_v, f) + EPS)
    W = np.asarray(qkv_w, f) * s[:, None]
    t = np.asarray(qkv_b, f) - np.asarray(qkv_m, f) * s
    # fold attention scale into k rows
    W[NH_KD:2 * NH_KD] *= SCALE
    t = t.copy()
    t[NH_KD:2 * NH_KD] *= SCALE
    qkv_wT = np.ascontiguousarray(W.T).astype(BF16)          # [384, 1536]
    qkv_bias = np.ascontiguousarray(t.reshape(12, 128))

    sd = np.asarray(dw_g, f) / np.sqrt(np.asarray(dw_v, f) + EPS)
    wd = np.asarray(dw_w, f)[:, 0] * sd[:, None, None]        # [256, 3, 3]
    td = np.asarray(dw_b, f) - np.asarray(dw_m, f) * sd
    # [2, 9, 128]: per chunk, per tap, per channel (diag matrices built on-chip)
    dw_wp = np.ascontiguousarray(
        wd.reshape(2, 128, 9).transpose(0, 2, 1)).astype(BF16)
    dw_bias = np.ascontiguousarray(td.reshape(2, 128))

    sp = np.asarray(proj_g, f) / np.sqrt(np.asarray(proj_v, f) + EPS)
    Wp = np.asarray(proj_w, f) * sp[:, None]
    tp = np.asarray(proj_b, f) - np.asarray(proj_m, f) * sp
    proj_wT = np.ascontiguousarray(Wp.T).astype(BF16)         # [1024, 384]
    proj_bias = np.ascontiguousarray(tp.reshape(3, 128))

    ab = np.asarray(attention_biases, f)[:, np.asarray(bias_idxs)]  # [8, 196, 196]
    ab = np.ascontiguousarray(np.exp(ab)).astype(BF16)

    x_bf = np.ascontiguousarray(x.reshape(B, DIM, N)).astype(BF16)
    return x_bf, dict(qkv_wT=qkv_wT, dw_w=dw_wp, proj_wT=proj_wT,
                      qkv_bias=qkv_bias, dw_bias=dw_bias, proj_bias=proj_bias, ab=ab)


def kernel(**inputs):
    global LAST_RESULT
    x_bf, consts = _prep_host(**inputs)
    nc = _get_nc()
    in_maps = []
    for c in range(NCORES):
        m = {"x": np.ascontiguousarray(x_bf[c * BPC:(c + 1) * BPC])}
        m.update(consts)
        in_maps.append(m)
    res = run_bass_kernel_spmd(nc, in_maps, core_ids=list(range(NCORES)))
    LAST_RESULT = res
    out = np.concatenate([r["out"] for r in res.results], axis=0)
    return np.ascontiguousarray(out.reshape(B, DIM, RES, RES)).astype(np.float32)
